# revision 1
# baseline (speedup 1.0000x reference)
"""Trainium2 Bass kernel for nn_BbVertLoss (point-in-bbox CE + IoU + L2 loss).

Strategy (pure data parallel, hardcoded for B=16, N=40960, H=24, 8 cores):
  - Each core gets 2 batches. Points live on partitions: partition p holds
    640 consecutive points of batch p//64 (local), laid out [128, 640*9].
  - Pred leg (real units, clip needs them):
      u_d = r_d^2 - (x_d-c_d)^2;  tcl = clip(max_d((x_d-c_d)^2 - r_d^2), +-.2)
      p = sigmoid(-100*tcl), accum -> S_p
  - GT leg (normalized units, only the sign matters):
      q_d = ((x_d-c'_d)/r'_d)^2 computed as Square(x*s + t) on ACT
      (per-partition scale AP!), combined with a stock bf16 TT MAX (2x mode),
      then one fused DVE op: g = (max(sq(x*s_x+t_x), m_yz) < 1), accum -> S_g
  - sel' = helper*|p + g - 1| + (1-helper)  (helper folded in so the Ln
    accumulation can batch across h in one group-wide instruction whose
    single accumulator directly yields the helper-weighted CE partial).
  - Ln per group of GRP h: one ACT op over [128, GRP*640], accum -> CE.
  - Host: partition+core reduction and final combine in f64.
  - Custom fused DVE ops registered into concourse.dve_ops:
      ANT_SUB2MAX:     max(in0-s0, in1-s1)
      ANT_SQMAXCLIP:   clip(max((in0-s0)^2-s1, in1), imm2, -imm2)
      ANT_SQMAXLT1SUM: (max((in0*s0+s1)^2, in1) < 1) [+ sum]
      ANT_SELHSUM:     s0*|in0 + (in1-1)| + s1 [+ sum]
      ANT_SQSUB / ANT_SQSUBMAX: DVE-route pred legs for engine balance
"""

import numpy as np

B, N, H = 16, 40960, 24
NCORES = 8
BPC = B // NCORES            # batches per core = 2
PPB = 64                     # partitions per batch
FPT = N // PPB               # points per partition = 640
NPART = BPC * PPB            # 128
RAWF = FPT * 3               # xyz de-interleaved on host: [x|y|z] per partition
import os
GRP = int(os.environ.get("KGRP", "8"))   # h-group size (ACT table amortization)
WBUFS = int(os.environ.get("KWBUFS", "4"))
NB = int(os.environ.get("KNB", "12"))     # of 24 h: pred legs on DVE route
SCW = 16                     # scal columns per (batch, h)
NGRP = (H + GRP - 1) // GRP

_CACHE = {}


def _register_custom_ops():
    """Register fused DVE ops in the module-level registries (idempotent)."""
    import concourse.dve_ops as dops
    from concourse.dve_spec import (Spec, Src0, Src1, C0, C1, C2, Zero, One,
                                    maxx, minn, sq, lower, AluOp)
    from concourse.dve_table_gen import dve_ver_for
    from concourse.dve_uop import DveOpSpec

    if "ANT_SUB2MAX" in dops._SUB_OPCODE_FOR_NAME:
        _CACHE["ops"] = {o.name: o for o in dops.OPS}
        return

    ver = dve_ver_for("TRN2")

    def ref_sub2max(in0, in1, s0, s1, imm2):
        return np.maximum(in0 - s0, in1 - s1)

    def ref_sqmaxclip(in0, in1, s0, s1, imm2):
        return np.minimum(np.maximum(np.maximum((in0 - s0) ** 2 - s1, in1),
                                     imm2), -imm2)

    def ref_sqmaxlt1sum(in0, in1, s0, s1, imm2):
        b = (np.maximum((in0 * s0 + s1) ** 2, in1) < 1.0).astype(np.float32)
        return b, b.reshape(b.shape[0], -1).sum(axis=-1, keepdims=True).astype(
            np.float32)

    def ref_selhsum(in0, in1, s0, s1, imm2):
        t = ((in1 - np.float32(1.0)) + in0).astype(np.float32)
        b = (s0 * np.abs(t) + s1).astype(np.float32)
        return b, b.reshape(b.shape[0], -1).sum(axis=-1, keepdims=True).astype(
            np.float32)

    def ref_sqsubmax(in0, in1, s0, s1, imm2):
        return np.maximum((in0 - s0) ** 2 - s1, in1)

    _t = Src0 + (Src1 - One)
    _q = sq(Src0 * C0 + C1)
    specs = [
        ("ANT_SUB2MAX", Spec(body=maxx(Src0 - C0, Src1 - C1),
                             reference=ref_sub2max)),
        ("ANT_SQMAXCLIP", Spec(body=minn(maxx(maxx(sq(Src0 - C0) - C1, Src1),
                                              C2), Zero - C2),
                               reference=ref_sqmaxclip)),
        ("ANT_SQMAXLT1SUM", Spec(body=(maxx(_q, Src1) < One),
                                 accum=AluOp.ADD, reference=ref_sqmaxlt1sum)),
        ("ANT_SELHSUM", Spec(body=maxx(_t, Zero - _t) * C0 + C1,
                             accum=AluOp.ADD, reference=ref_selhsum)),
        ("ANT_SQSUBMAX", Spec(body=maxx(sq(Src0 - C0) - C1, Src1),
                              reference=ref_sqsubmax)),
        ("ANT_SQSUB", Spec(body=sq(Src0 - C0) - C1,
                           reference=lambda in0, in1, s0, s1, imm2:
                               (in0 - s0) ** 2 - s1)),
    ]
    for name, spec in specs:
        opcode = max(dops._SUB_OPCODE_FOR_NAME.values()) + 1
        assert opcode < 0x20
        tmp = DveOpSpec(name=name, opcode=opcode, uops=lower(spec, ver=ver),
                        rd1_en=True)
        op = dops.DveOp(name, spec, subdim=False, uops_sha={ver: tmp.sha(ver)})
        dops.OPS.append(op)
        dops.CUSTOM_DVE_SPECS[name] = spec
        dops._SUB_OPCODE_FOR_NAME[name] = opcode
    _CACHE["ops"] = {o.name: o for o in dops.OPS}


def _build_module():
    import concourse.bacc as bacc
    import concourse.tile as tile
    from concourse import mybir

    _register_custom_ops()
    OPS = _CACHE["ops"]

    f32 = mybir.dt.float32
    bf16 = mybir.dt.bfloat16
    fp16 = mybir.dt.float16
    Act = mybir.ActivationFunctionType
    Alu = mybir.AluOpType

    # pred h's routed to all-DVE legs: the whole LAST group goes DVE so the
    # Scalar tail (last sigmoids + ln) shrinks while Vector's end-idle absorbs
    # it; the remainder spreads over the earlier h's.
    dve_hs = set(int(round(i * H / max(NB, 1))) for i in range(NB)) if NB else set()

    nc = bacc.Bacc("TRN2", debug=False)

    xpc = nc.dram_tensor("xpc", [NPART, RAWF], f32, kind="ExternalInput")
    scal = nc.dram_tensor("scal", [NPART, H * SCW], f32, kind="ExternalInput")
    accP_d = nc.dram_tensor("accP", [NPART, H], f32, kind="ExternalOutput")
    accG_d = nc.dram_tensor("accG", [NPART, H], f32, kind="ExternalOutput")
    accS_d = nc.dram_tensor("accS", [NPART, H], f32, kind="ExternalOutput")
    accL_d = nc.dram_tensor("accL", [NPART, 2 * NGRP], f32, kind="ExternalOutput")

    with tile.TileContext(nc) as tc:
        with (
            tc.tile_pool(name="data", bufs=1) as data,
            tc.tile_pool(name="work", bufs=WBUFS) as work,
            tc.tile_pool(name="phase", bufs=6) as phase,
            tc.tile_pool(name="selp", bufs=2) as selp,
            tc.tile_pool(name="pp", bufs=4, space="PSUM") as pp,
        ):
            sc = data.tile([NPART, H * SCW], f32, tag="sc")
            # split per h-group so group 0's constants land first
            for c0 in range(0, H * SCW, 8 * SCW):
                c1 = min(c0 + 8 * SCW, H * SCW)
                nc.sync.dma_start(out=sc[:, c0:c1], in_=scal[:, c0:c1])
            raw = data.tile([NPART, RAWF], f32, tag="raw")
            for dord in (1, 2, 0):   # y first (consumed first), x last
                cs = slice(FPT * dord, FPT * (dord + 1))
                for half in range(2):
                    p0, p1 = 64 * half, 64 * (half + 1)
                    nc.sync.dma_start(out=raw[p0:p1, cs], in_=xpc[p0:p1, cs])
            eps8 = data.tile([NPART, 1], f32, tag="eps8")
            nc.vector.memset(eps8[:], 1e-8)
            warm = data.tile([NPART, 1], f32, tag="warm")
            nc.scalar.activation(warm[:], eps8[:], Act.Sigmoid,
                                 bias=0.0, scale=-100.0)

            accP = data.tile([NPART, H], f32, tag="accP")
            accG = data.tile([NPART, H], f32, tag="accG")
            accS = data.tile([NPART, H], f32, tag="accS")
            accL = data.tile([NPART, 2 * NGRP], f32, tag="accL")
            nc.vector.memset(accL[:], 0.0)

            xs = [raw[:, FPT * d : FPT * (d + 1)] for d in range(3)]

            def col(h, j):
                return sc[:, SCW * h + j : SCW * h + j + 1]

            gsizes = [int(v) for v in os.environ.get(
                "KGL", "8,8,8").split(",")]
            assert sum(gsizes) == H
            gstarts = [sum(gsizes[:i]) for i in range(len(gsizes))]
            QUAD = int(os.environ.get("KQUAD", "1"))
            for gi, (h0, ng) in enumerate(zip(gstarts, gsizes)):
                selbig = selp.tile([NPART, ng * FPT], bf16, tag="selbig")
                for q0 in range(h0, h0 + ng, QUAD):
                    qh = range(q0, min(q0 + QUAD, h0 + ng))
                    nq = len(qh)
                    # gt y/z squares of the quad land in one wide tile so a
                    # single bf16 TT MAX (2x mode, no per-h constants) serves
                    # all nq h at once
                    qgy = work.tile([NPART, nq * FPT], bf16, tag="qgy")
                    qgz = work.tile([NPART, nq * FPT], bf16, tag="qgz")
                    myz = work.tile([NPART, nq * FPT], bf16, tag="myz")
                    tcls = {}
                    for h in qh:
                        # pred leg: tcl = clip(max_d((x-c)^2 - r^2), +-0.2)
                        if h in dve_hs:
                            qsy = work.tile([NPART, FPT], f32, tag="qsy")
                            nc.vector._custom_dve(OPS["ANT_SQSUB"],
                                                  out=qsy[:], in0=xs[1],
                                                  s0=col(h, 6), s1=col(h, 2))
                            m1 = work.tile([NPART, FPT], f32, tag="m1")
                            nc.vector._custom_dve(OPS["ANT_SQSUBMAX"],
                                                  out=m1[:], in0=xs[2],
                                                  in1=qsy[:],
                                                  s0=col(h, 7), s1=col(h, 3))
                        else:
                            sqy = work.tile([NPART, FPT], f32, tag="sqy")
                            nc.scalar.activation(sqy[:], xs[1], Act.Square,
                                                 bias=col(h, 0), scale=1.0)
                            sqz = work.tile([NPART, FPT], f32, tag="sqz")
                            nc.scalar.activation(sqz[:], xs[2], Act.Square,
                                                 bias=col(h, 1), scale=1.0)
                            m1 = work.tile([NPART, FPT], f32, tag="m1")
                            nc.vector._custom_dve(OPS["ANT_SUB2MAX"],
                                                  out=m1[:], in0=sqy[:],
                                                  in1=sqz[:],
                                                  s0=col(h, 2), s1=col(h, 3))
                        tcl = phase.tile([NPART, FPT], f32, tag="tcl")
                        nc.vector._custom_dve(OPS["ANT_SQMAXCLIP"],
                                              out=tcl[:], in0=xs[0],
                                              in1=m1[:], s0=col(h, 4),
                                              s1=col(h, 5), imm2=-0.2)
                        tcls[h] = tcl
                        jq = h - q0
                        qs = slice(jq * FPT, (jq + 1) * FPT)
                        nc.scalar.activation(qgy[:, qs], xs[1], Act.Square,
                                             bias=col(h, 9), scale=col(h, 8))
                        nc.scalar.activation(qgz[:, qs], xs[2], Act.Square,
                                             bias=col(h, 11), scale=col(h, 10))
                    nc.vector.tensor_tensor(out=myz[:], in0=qgy[:],
                                            in1=qgz[:], op=Alu.max)
                    for h in qh:
                        jq = h - q0
                        ms = slice(jq * FPT, (jq + 1) * FPT)
                        g = phase.tile([NPART, FPT], bf16, tag="g")
                        nc.vector._custom_dve(OPS["ANT_SQMAXLT1SUM"],
                                              out=g[:], in0=xs[0],
                                              in1=myz[:, ms],
                                              s0=col(h, 12), s1=col(h, 13),
                                              accum_out=accG[:, h : h + 1])
                        p = pp.tile([NPART, FPT], f32, tag="p")
                        nc.scalar.activation(p[:], tcls[h][:], Act.Sigmoid,
                                             bias=0.0, scale=-100.0,
                                             accum_out=accP[:, h : h + 1])
                        j = h - h0
                        nc.vector._custom_dve(
                            OPS["ANT_SELHSUM"],
                            out=selbig[:, j * FPT : (j + 1) * FPT],
                            in0=p[:], in1=g[:],
                            s0=col(h, 14), s1=col(h, 15),
                            accum_out=accS[:, h : h + 1])
                # one batched Ln for the whole group; its accumulator IS the
                # helper-weighted CE partial sum for these h
                lnout = selp.tile([NPART, ng * FPT], bf16, tag="lnout")
                nc.scalar.activation(lnout[:], selbig[:], Act.Ln,
                                     bias=eps8[:], scale=1.0,
                                     accum_out=accL[:, 2 * gi : 2 * gi + 1])

            nc.sync.dma_start(out=accP_d[:], in_=accP[:])
            nc.sync.dma_start(out=accG_d[:], in_=accG[:])
            nc.sync.dma_start(out=accS_d[:], in_=accS[:])
            nc.sync.dma_start(out=accL_d[:], in_=accL[:])

    nc.compile()
    return nc


def _get_module():
    if "nc" not in _CACHE:
        _CACHE["nc"] = _build_module()
    return _CACHE["nc"]


def _make_inputs(X_pc, y_bbvert_pred, Y_bbvert):
    """Build per-core input maps (host-side shard + scalar precompute)."""
    X_pc = np.ascontiguousarray(X_pc, dtype=np.float32)
    pred = np.asarray(y_bbvert_pred, dtype=np.float32)
    gt = np.asarray(Y_bbvert, dtype=np.float32)

    helper = (gt.reshape(B, H, 6).sum(axis=-1) > 0.0).astype(np.float32)

    def params(t):
        a = t[:, :, 0, :]
        b = t[:, :, 1, :]
        c = ((a + b) * np.float32(0.5)).astype(np.float32)
        r = ((b - a) * np.float32(0.5)).astype(np.float32)
        return c, r

    c, r = params(pred)
    rsq = (r * r).astype(np.float32)
    cg, rg = params(gt)
    # guarded reciprocal half-width for the normalized gt test
    rg_abs = np.maximum(np.abs(rg), np.float32(1e-7))
    s = (1.0 / rg_abs).astype(np.float32)
    t = (-cg * s).astype(np.float32)

    cols = [-c[:, :, 1], -c[:, :, 2], rsq[:, :, 1], rsq[:, :, 2],
            c[:, :, 0], rsq[:, :, 0], c[:, :, 1], c[:, :, 2],
            s[:, :, 1], t[:, :, 1], s[:, :, 2], t[:, :, 2],
            s[:, :, 0], t[:, :, 0], helper, 1.0 - helper]
    sc_all = np.stack(cols, axis=-1).astype(np.float32)  # [B,H,16]

    in_maps = []
    for k in range(NCORES):
        rows = []
        scs = []
        for b in range(BPC):
            bi = BPC * k + b
            # de-interleave xyz on host: partition row = [x(640)|y(640)|z(640)]
            xyz = X_pc[bi].reshape(PPB, FPT, 9)[:, :, :3]
            rows.append(xyz.transpose(0, 2, 1).reshape(PPB, RAWF))
            scs.append(np.broadcast_to(sc_all[bi][None], (PPB, H, SCW)))
        in_maps.append({
            "xpc": np.ascontiguousarray(np.concatenate(rows, axis=0)),
            "scal": np.ascontiguousarray(
                np.concatenate(scs, axis=0).reshape(NPART, H * SCW)),
        })
    return in_maps


def _combine(results, y_bbvert_pred, Y_bbvert):
    """Host-side: partition+core reduction and final loss combine (f64)."""
    pred = np.asarray(y_bbvert_pred, dtype=np.float32)
    gt = np.asarray(Y_bbvert, dtype=np.float32)

    helper = (gt.reshape(B, H, 6).sum(axis=-1) > 0.0).astype(np.float64)

    Sp = np.zeros((B, H)); Sg = np.zeros((B, H)); Ssh = np.zeros((B, H))
    Sln = 0.0
    for k in range(NCORES):
        r = results[k]
        Sln += r["accL"].astype(np.float64).sum()
        for b in range(BPC):
            bi = BPC * k + b
            sl = slice(PPB * b, PPB * (b + 1))
            Sp[bi] = r["accP"][sl].astype(np.float64).sum(axis=0)
            Sg[bi] = r["accG"][sl].astype(np.float64).sum(axis=0)
            Ssh[bi] = r["accS"][sl].astype(np.float64).sum(axis=0)

    # undo the helper fold: Ssh = helper*S_sel + (1-helper)*N
    Ss = np.where(helper > 0, Ssh, 0.0)
    Tp = (Ss + Sg + Sp - float(N)) * 0.5

    denom_ce = helper.sum() * N
    loss_ce = -Sln / denom_ce

    den = np.where(helper > 0, Sp + Sg - Tp + 1e-6, 1.0)
    iou_all = -(Tp / den)
    loss_iou = (iou_all * helper).sum() / helper.sum()

    l2_all = ((gt.astype(np.float64) - pred.astype(np.float64)) ** 2
              ).reshape(B, H, 6).mean(axis=-1)
    l2_pos = (l2_all * helper).sum() / helper.sum()
    negw = (1.0 - helper)[:, :, None]
    dneg = (pred[:, :, 0, :].astype(np.float64) - pred[:, :, 1, :].astype(np.float64))
    l2_neg = ((negw * dneg) ** 2).sum() / ((1.0 - helper).sum() + 1e-8)
    loss_l2 = l2_pos + l2_neg

    total = loss_ce + loss_l2 + loss_iou
    return (np.float32(total), np.float32(loss_l2),
            np.float32(loss_ce), np.float32(loss_iou))


def run(X_pc, y_bbvert_pred, Y_bbvert, trace=False):
    from concourse.bass_utils import run_bass_kernel_spmd

    nc = _get_module()
    in_maps = _make_inputs(X_pc, y_bbvert_pred, Y_bbvert)
    res = run_bass_kernel_spmd(nc, in_maps, core_ids=list(range(NCORES)),
                               trace=trace)
    out = _combine(res.results, y_bbvert_pred, Y_bbvert)
    return out, res


def kernel(X_pc, y_bbvert_pred, Y_bbvert):
    out, _ = run(X_pc, y_bbvert_pred, Y_bbvert, trace=False)
    return out



# revision 2
# speedup vs baseline: 1.0060x; 1.0060x over previous
"""Trainium2 Bass kernel v2 for nn_BbVertLoss — wide-batched redesign.

Layout (per core, 2 batches, H=24): partition p = hs*16 + b*8 + g where
hs in [0,8) = h-slot, b in [0,2) = local batch, g in [0,8) = point group.
Each partition holds FD=5120 consecutive points (of batch b, group g),
replicated across the 8 h-slots. Group gi processes h = gi*8 + hs for
all partitions at once -> every op is one wide [128, 5120] pass.

Math (per point, h):  u = max_d (x_d-a_d)(x_d-b_d)  [pred corners]
  qmax likewise for gt corners; g = (qmax < 0)
  p = sigmoid(-100 u);  sgn = (qmax>=0) - 0.5;  m = sgn * u
  sel = sigmoid(200 m + BIG*(1-helper))   [= g?p:1-p, helper-folded]
  CE  = -sum ln(sel + 1e-8); Sp = sum p; Sg = sum g; Ssel = sum sel
  TP = (Ssel + Sg + Sp - N)/2; iou = -TP/(Sp+Sg-TP+1e-6); l2 on host.

Vector: 6x custom ANT_PRODMAX at 2X_1PORT (hand-written uops, T1),
        1x stock TS (sgn, 4x), 1x stock TT (m, 2x) per group.
Scalar: 3 batched ACTs/group (sigmoid x2 + sign) with per-partition
        accum_out, then ONE table switch + one [128,15360] Ln.
"""

import numpy as np

B, N, H = 16, 40960, 24
NCORES = 8
BPC = 2                      # batches per core
HS = 8                       # h-slots (h per group)
NG = H // HS                 # 3 groups
GP = 8                       # point groups
FD = N // GP                 # 5120 points per partition
NPART = HS * BPC * GP        # 128
SCW = 16                     # const cols per group
BIGNEG = -60000.0

_CACHE = {}


# --------------------------------------------------------------------------
# T1: custom ANT_PRODMAX with hand-written 2X_1PORT uop program
# --------------------------------------------------------------------------

def _register_prodmax():
    import concourse.dve_ops as dops
    from concourse.dve_spec import (Spec, Src0, Src1, C0, C1, Zero, maxx,
                                    select, lower)
    from concourse.dve_uop import (
        UopConfig, DveOpSpec, AluOp, AluInp, DelayInp, InpSel, OutSel,
        OutPath, Trigger, ENABLE,
    )
    from concourse.dve_table_gen import dve_ver_for

    ver = dve_ver_for("TRN2")
    if "ANT_PRODMAX" in dops._SUB_OPCODE_FOR_NAME:
        _CACHE["op"] = next(o for o in dops.OPS if o.name == "ANT_PRODMAX")
        _CACHE["msel"] = next(o for o in dops.OPS if o.name == "ANT_MSEL")
        return

    spec = Spec(
        body=maxx((Src0 - C0) * (Src0 - C1), Src1),
        reference=lambda in0, in1, s0, s1, imm2: np.maximum(
            (in0 - s0) * (in0 - s1), in1).astype(np.float32),
    )

    u = UopConfig()
    u.enable_input(InpSel.SRC_0, 0)
    u.enable_input(InpSel.CONST_0, 1)   # L0
    u.enable_input(InpSel.CONST_1, 2)   # L1
    u.enable_input(InpSel.SRC_0_HI, 3)  # L2
    u.enable_input(InpSel.SRC_1, 4)     # L3
    u.enable_input(InpSel.SRC_1_HI, 5)  # L4
    u.require_inp0 = ENABLE
    u.require_inp1 = ENABLE
    u.trigger = (Trigger.SRC_TENSOR_DONE, Trigger.NONE, Trigger.NONE)
    dp = u.datapath_config
    dp[0].enable_alu(AluOp.SUBTRACT, AluInp.PREV_ALU_OUT, AluInp.PREV_DELAY_0)
    dp[0].enable_delay_from_src(DelayInp.PREV_ALU_OUT, 5)
    dp[0].pass_through_delay(0, 1, 2, 3, 4)
    dp[1].enable_alu(AluOp.SUBTRACT, AluInp.PREV_DELAY_5, AluInp.PREV_DELAY_1)
    dp[1].enable_delay_from_src(DelayInp.PREV_ALU_OUT, 5)
    dp[1].pass_through_delay(0, 1, 2, 3, 4)
    dp[2].enable_alu(AluOp.MULTIPLY, AluInp.PREV_ALU_OUT, AluInp.PREV_DELAY_5)
    dp[2].pass_through_delay(0, 1, 2, 3, 4)
    dp[3].enable_alu(AluOp.MAX, AluInp.PREV_ALU_OUT, AluInp.PREV_DELAY_3)
    dp[3].pass_through_delay(0, 1, 2, 4)
    dp[4].enable_alu(AluOp.SUBTRACT, AluInp.PREV_DELAY_2, AluInp.PREV_DELAY_0)
    dp[4].enable_delay_from_src(DelayInp.PREV_ALU_OUT, 3)
    dp[4].pass_through_delay(1, 2, 4)
    dp[5].enable_alu(AluOp.SUBTRACT, AluInp.PREV_DELAY_2, AluInp.PREV_DELAY_1)
    dp[5].enable_delay_from_src(DelayInp.PREV_ALU_OUT, 0)
    dp[5].pass_through_delay(3, 4)
    dp[6].enable_alu(AluOp.MULTIPLY, AluInp.PREV_ALU_OUT, AluInp.PREV_DELAY_0)
    dp[6].pass_through_delay(3, 4)
    dp[7].enable_alu(AluOp.MAX, AluInp.PREV_ALU_OUT, AluInp.PREV_DELAY_4)
    dp[7].pass_through_delay(3)
    u.enable_output(OutSel.DELAY_3, OutPath.WR0_LO)
    u.enable_output(OutSel.ALU_OUT, OutPath.WR0_HI)
    u.validate(ver)

    # ---- ANT_MSEL: out = (Src1 < 0) ? -Src0 : Src0  (2x program) ----
    msel_spec = Spec(
        body=select(Src1 < Zero, Zero - Src0, Src0),
        reference=lambda in0, in1, s0, s1, imm2: np.where(
            in1 < 0, -in0, in0).astype(np.float32),
    )
    um = UopConfig()
    um.enable_input(InpSel.SRC_1, 1)     # L0 = qmax_lo
    um.enable_input(InpSel.ZERO, 2)      # L1 = 0
    um.enable_input(InpSel.SRC_0, 3)     # L2 = u_lo
    um.enable_input(InpSel.SRC_0_HI, 4)  # L3 = u_hi
    um.enable_input(InpSel.SRC_1_HI, 5)  # L4 = qmax_hi
    um.require_inp0 = ENABLE
    um.require_inp1 = ENABLE
    um.trigger = (Trigger.SRC_TENSOR_DONE, Trigger.NONE, Trigger.NONE)
    dm = um.datapath_config
    # elem0: cond, neg, shim, select
    dm[0].enable_alu(AluOp.IS_LT, AluInp.PREV_DELAY_0, AluInp.PREV_DELAY_1)
    dm[0].pass_through_delay(1, 2, 3, 4)
    dm[1].enable_alu(AluOp.SUBTRACT, AluInp.PREV_DELAY_1, AluInp.PREV_DELAY_2)
    dm[1].enable_delay_from_src(DelayInp.PREV_ALU_OUT, 0)  # cond0
    dm[1].pass_through_delay(1, 2, 3, 4)
    dm[2].enable_alu(AluOp.IS_NE, AluInp.PREV_DELAY_0, AluInp.PREV_DELAY_1)
    dm[2].enable_delay_from_src(DelayInp.PREV_ALU_OUT, 0)  # neg0
    dm[2].pass_through_delay(1, 2, 3, 4)
    dm[3].enable_alu(AluOp.SELECT, AluInp.PREV_DELAY_2, AluInp.PREV_DELAY_0)
    dm[3].pass_through_delay(1, 3, 4)
    # elem1
    dm[4].enable_alu(AluOp.IS_LT, AluInp.PREV_DELAY_4, AluInp.PREV_DELAY_1)
    dm[4].enable_delay_from_src(DelayInp.PREV_ALU_OUT, 0)  # sel0
    dm[4].pass_through_delay(1, 3)
    dm[5].enable_alu(AluOp.SUBTRACT, AluInp.PREV_DELAY_1, AluInp.PREV_DELAY_3)
    dm[5].enable_delay_from_src(DelayInp.PREV_ALU_OUT, 4)  # cond1
    dm[5].pass_through_delay(0, 1, 3)
    dm[6].enable_alu(AluOp.IS_NE, AluInp.PREV_DELAY_4, AluInp.PREV_DELAY_1)
    dm[6].enable_delay_from_src(DelayInp.PREV_ALU_OUT, 4)  # neg1
    dm[6].pass_through_delay(0, 3)
    dm[7].enable_alu(AluOp.SELECT, AluInp.PREV_DELAY_3, AluInp.PREV_DELAY_4)
    dm[7].pass_through_delay(0)
    um.enable_output(OutSel.DELAY_0, OutPath.WR0_LO)
    um.enable_output(OutSel.ALU_OUT, OutPath.WR0_HI)
    um.validate(ver)

    for name, sp, u1x, u2x in (
        ("ANT_PRODMAX", spec, lower(spec, ver=ver), [u]),
        ("ANT_MSEL", msel_spec, lower(msel_spec, ver=ver), [um]),
    ):
        opcode = max(dops._SUB_OPCODE_FOR_NAME.values()) + 1
        assert opcode < 0x20
        compiled = DveOpSpec(name=name, opcode=opcode, uops=u1x,
                             uops_2x=u2x, rd1_en=True, perf_max=1)
        op = dops.DveOp(name, sp, subdim=False,
                        uops_sha={ver: compiled.sha(ver)})
        dops.OPS.append(op)
        dops.CUSTOM_DVE_SPECS[name] = sp
        dops._SUB_OPCODE_FOR_NAME[name] = opcode
        dops._COMPILE_CACHE[(name, ver)] = compiled
    _CACHE["op"] = next(o for o in dops.OPS if o.name == "ANT_PRODMAX")
    _CACHE["msel"] = next(o for o in dops.OPS if o.name == "ANT_MSEL")


def _cust2x(vec, op, *, out, in0, in1, s0=0.0, s1=0.0):
    """Emit a custom op with perf_max=1 (2X_1PORT for fp16 step-1 APs)."""
    inst = vec._custom_dve(op, out=out, in0=in0, in1=in1, s0=s0, s1=s1)
    mi = getattr(inst, "ins", None) or inst
    mi.perf_max = 1
    return inst


def _prodmax(vec, *, out, in0, in1, s0, s1):
    return _cust2x(vec, _CACHE["op"], out=out, in0=in0, in1=in1, s0=s0, s1=s1)


# --------------------------------------------------------------------------
# module build
# --------------------------------------------------------------------------

def _build_module():
    import concourse.bacc as bacc
    import concourse.tile as tile
    from concourse import mybir

    _register_prodmax()

    f32 = mybir.dt.float32
    fp16 = mybir.dt.float16
    Act = mybir.ActivationFunctionType
    Alu = mybir.AluOpType

    nc = bacc.Bacc("TRN2", debug=False)

    # planes: [x | y | z] each FD wide, fp16 (host pre-converted)
    xyz = nc.dram_tensor("xyz", [NPART, 3 * FD], fp16, kind="ExternalInput")
    scal = nc.dram_tensor("scal", [NPART, NG * SCW], f32, kind="ExternalInput")
    acc_d = nc.dram_tensor("acc", [NPART, 32], f32, kind="ExternalOutput")

    with tile.TileContext(nc) as tc:
        with (
            tc.tile_pool(name="data", bufs=1) as data,
            tc.tile_pool(name="work", bufs=1) as work,
            tc.tile_pool(name="mwork", bufs=2) as mwork,
            tc.tile_pool(name="mwb", bufs=1) as mwb,
            tc.tile_pool(name="psc", bufs=1) as psc,
        ):
            negt = data.tile([NPART, FD], fp16, tag="neg")
            nc.vector.memset(negt[:], BIGNEG)
            neg = negt[:]
            sc = data.tile([NPART, NG * SCW], f32, tag="sc")
            nc.sync.dma_start(out=sc[:], in_=scal[:])
            # separate tiles per plane -> per-plane DMA dependencies;
            # 4 chunks/plane round-robined over the 3 DMA queues, y first
            planes = []
            for d in range(3):
                pln = data.tile([NPART, FD], fp16, tag=f"pl{d}", name=f"pl{d}")
                planes.append(pln)
            for d in (1, 2, 0):
                cs = slice(FD * d, FD * (d + 1))
                nc.sync.dma_start(out=planes[d][:], in_=xyz[:, cs])
            acc = data.tile([NPART, 32], f32, tag="acc")
            nc.vector.memset(acc[:], 0.0)
            eps8 = data.tile([NPART, 1], f32, tag="eps8")
            nc.vector.memset(eps8[:], 1e-8)
            selbig = data.tile([NPART, NG * FD], fp16, tag="selbig")
            # warm the sigmoid table before first use
            warm = data.tile([NPART, 1], f32, tag="warm")
            nc.scalar.activation(warm[:], eps8[:], Act.Sigmoid,
                                 bias=0.0, scale=-100.0)

            xs = [planes[d][:] for d in range(3)]

            def col(gi, j):
                return sc[:, SCW * gi + j: SCW * gi + j + 1]

            for gi in range(NG):
                # pred leg: u = max over dims of (x-a)(x-b)
                uy = work.tile([NPART, FD], fp16, tag="uy")
                _prodmax(nc.vector, out=uy[:], in0=xs[1], in1=neg,
                         s0=col(gi, 0), s1=col(gi, 1))
                uyz = work.tile([NPART, FD], fp16, tag="uyz")
                _prodmax(nc.vector, out=uyz[:], in0=xs[2], in1=uy[:],
                         s0=col(gi, 2), s1=col(gi, 3))
                u = mwork.tile([NPART, FD], fp16, tag="u")
                _prodmax(nc.vector, out=u[:], in0=xs[0], in1=uyz[:],
                         s0=col(gi, 4), s1=col(gi, 5))
                # p = sigmoid(-100 u), accum -> Sp
                ps = psc.tile([NPART, FD], fp16, tag="scratch")
                nc.scalar.activation(ps[:], u[:], Act.Sigmoid, bias=0.0,
                                     scale=-100.0,
                                     accum_out=acc[:, 3 * gi + 1: 3 * gi + 2])
                # gt leg
                qy = work.tile([NPART, FD], fp16, tag="qy")
                _prodmax(nc.vector, out=qy[:], in0=xs[1], in1=neg,
                         s0=col(gi, 6), s1=col(gi, 7))
                qyz = work.tile([NPART, FD], fp16, tag="qyz")
                _prodmax(nc.vector, out=qyz[:], in0=xs[2], in1=qy[:],
                         s0=col(gi, 8), s1=col(gi, 9))
                qmax = mwork.tile([NPART, FD], fp16, tag="qmax")
                _prodmax(nc.vector, out=qmax[:], in0=xs[0], in1=qyz[:],
                         s0=col(gi, 10), s1=col(gi, 11))
                # m = (qmax < 0) ? -u : u   (= (1-2g) * u)
                m = mwb.tile([NPART, FD], fp16, tag="m")
                _cust2x(nc.vector, _CACHE["msel"], out=m[:], in0=u[:],
                        in1=qmax[:])
                if gi < NG - 1:
                    # sign(qmax), accum -> Ssign (Scalar)
                    sg = psc.tile([NPART, FD], fp16, tag="scratch")
                    nc.scalar.activation(sg[:], qmax[:], Act.Sign, bias=0.0,
                                         scale=1.0,
                                         accum_out=acc[:, 3 * gi + 2:
                                                       3 * gi + 3])
                    # sel = sigmoid(100 m + BIG*(1-helper)) -> selbig slice
                    nc.scalar.activation(selbig[:, gi * FD:(gi + 1) * FD],
                                         m[:], Act.Sigmoid, bias=col(gi, 12),
                                         scale=100.0,
                                         accum_out=acc[:, 3 * gi: 3 * gi + 1])
                else:
                    # last group: Sg via TS is_lt accum on Vector (1x; hides
                    # in V's tail shadow while Scalar runs sel/ln)
                    gq = psc.tile([NPART, FD], fp16, tag="scratch2")
                    nc.vector.tensor_scalar(out=gq[:], in0=qmax[:],
                                            scalar1=0.0, scalar2=0.0,
                                            op0=Alu.is_lt, op1=Alu.add,
                                            accum_out=acc[:, 3 * gi + 2:
                                                          3 * gi + 3])
                    mlast = m

            # Ln for groups 0..NG-2 while V finishes; then the last group's
            # sel sigmoid; then its Ln.
            for gi in range(NG - 1):
                lo = psc.tile([NPART, FD], fp16, tag="scratch",
                              name=f"ln{gi}")
                nc.scalar.activation(lo[:], selbig[:, gi * FD:(gi + 1) * FD],
                                     Act.Ln, bias=eps8[:], scale=1.0,
                                     accum_out=acc[:, 9 + gi: 10 + gi])
            gi = NG - 1
            nc.scalar.activation(selbig[:, gi * FD:(gi + 1) * FD],
                                 mlast[:], Act.Sigmoid, bias=col(gi, 12),
                                 scale=100.0,
                                 accum_out=acc[:, 3 * gi: 3 * gi + 1])
            lo2 = psc.tile([NPART, FD], fp16, tag="scratch", name="ln2")
            nc.scalar.activation(lo2[:], selbig[:, gi * FD:(gi + 1) * FD],
                                 Act.Ln, bias=eps8[:], scale=1.0,
                                 accum_out=acc[:, 9 + gi: 10 + gi])

            nc.sync.dma_start(out=acc_d[:], in_=acc[:])

    nc.compile()
    return nc


def _get_module():
    if "nc" not in _CACHE:
        _CACHE["nc"] = _build_module()
    return _CACHE["nc"]


# --------------------------------------------------------------------------
# host-side shard / combine
# --------------------------------------------------------------------------

def _make_inputs(X_pc, y_bbvert_pred, Y_bbvert):
    X_pc = np.asarray(X_pc, dtype=np.float32)
    pred = np.asarray(y_bbvert_pred, dtype=np.float32)
    gt = np.asarray(Y_bbvert, dtype=np.float32)

    helper = (gt.reshape(B, H, 6).sum(axis=-1) > 0.0).astype(np.float32)
    BIG = 50000.0

    in_maps = []
    for k in range(NCORES):
        # planes [128, 4*FD] fp16: partition p = hs*16 + b*8 + g
        xp = X_pc[2 * k: 2 * k + 2, :, :3].astype(np.float16)  # [2, N, 3]
        arr = xp.reshape(BPC, GP, FD, 3)                        # [b, g, FD, 3]
        # [hs, b, g, FD, 3] -> [128, FD, 3]
        rep = np.broadcast_to(arr[None], (HS, BPC, GP, FD, 3))
        rep = rep.reshape(NPART, FD, 3)
        planes = np.ascontiguousarray(
            rep.transpose(2, 0, 1).reshape(3, NPART, FD)
            .transpose(1, 0, 2).reshape(NPART, 3 * FD))

        # consts [128, NG*SCW] f32; partition-dependent via (hs, b)
        scal = np.zeros((NPART, NG * SCW), dtype=np.float32)
        for hs in range(HS):
            for b in range(BPC):
                bi = 2 * k + b
                rows = slice(hs * 16 + b * 8, hs * 16 + b * 8 + 8)
                for gi in range(NG):
                    h = gi * HS + hs
                    c = np.zeros(SCW, dtype=np.float32)
                    # pred corners y,z,x
                    c[0], c[1] = pred[bi, h, 0, 1], pred[bi, h, 1, 1]
                    c[2], c[3] = pred[bi, h, 0, 2], pred[bi, h, 1, 2]
                    c[4], c[5] = pred[bi, h, 0, 0], pred[bi, h, 1, 0]
                    # gt corners y,z,x
                    c[6], c[7] = gt[bi, h, 0, 1], gt[bi, h, 1, 1]
                    c[8], c[9] = gt[bi, h, 0, 2], gt[bi, h, 1, 2]
                    c[10], c[11] = gt[bi, h, 0, 0], gt[bi, h, 1, 0]
                    c[12] = BIG * (1.0 - helper[bi, h])  # sigmoid bias fold
                    scal[rows, gi * SCW:(gi + 1) * SCW] = c
        in_maps.append({"xyz": planes, "scal": scal})
    return in_maps


def _combine(results, y_bbvert_pred, Y_bbvert):
    pred = np.asarray(y_bbvert_pred, dtype=np.float64)
    gt = np.asarray(Y_bbvert, dtype=np.float64)
    helper = (gt.reshape(B, H, 6).sum(axis=-1) > 0.0).astype(np.float64)

    Ssel = np.zeros((B, H)); Sp = np.zeros((B, H)); Sg = np.zeros((B, H))
    CEsum = 0.0
    for k in range(NCORES):
        a = results[k]["acc"].astype(np.float64)  # [128, 32]
        CEsum += a[:, 9:12].sum()
        for hs in range(HS):
            for b in range(BPC):
                bi = 2 * k + b
                rows = slice(hs * 16 + b * 8, hs * 16 + b * 8 + 8)
                for gi in range(NG):
                    h = gi * HS + hs
                    Ssel[bi, h] += a[rows, 3 * gi + 0].sum()
                    Sp[bi, h] += a[rows, 3 * gi + 1].sum()
                    sgn_sum = a[rows, 3 * gi + 2].sum()
                    if gi < NG - 1:
                        Sg[bi, h] += (N - sgn_sum) / 2.0 - 0.0
                    else:
                        Sg[bi, h] += sgn_sum
    # note: for gi<NG-1 the count inside (N - sum)/2 uses N=40960 because
    # each (b,h) has 8 partitions x 5120 points summed above.

    # helper==0 rows: sel was forced to 1 -> Ssel = N; mask for iou anyway
    TP = (Ssel + Sg + Sp - N) / 2.0
    U = Sp + Sg - TP
    denom_ce = helper.sum() * N
    loss_ce = -CEsum / denom_ce

    iou_all = -(TP / (U + 1e-6))
    loss_iou = (iou_all * helper).sum() / helper.sum()

    l2_all = ((gt - pred) ** 2).reshape(B, H, 6).mean(axis=-1)
    l2_pos = (l2_all * helper).sum() / helper.sum()
    negw = (1.0 - helper)[:, :, None]
    dneg = pred[:, :, 0, :] - pred[:, :, 1, :]
    l2_neg = ((negw * dneg) ** 2).sum() / ((1.0 - helper).sum() + 1e-8)
    loss_l2 = l2_pos + l2_neg

    total = loss_ce + loss_l2 + loss_iou
    return (np.float32(total), np.float32(loss_l2),
            np.float32(loss_ce), np.float32(loss_iou))


def run(X_pc, y_bbvert_pred, Y_bbvert, trace=False):
    from concourse.bass_utils import run_bass_kernel_spmd

    nc = _get_module()
    in_maps = _make_inputs(X_pc, y_bbvert_pred, Y_bbvert)
    res = run_bass_kernel_spmd(nc, in_maps, core_ids=list(range(NCORES)),
                               trace=trace)
    out = _combine(res.results, y_bbvert_pred, Y_bbvert)
    return out, res


def kernel(X_pc, y_bbvert_pred, Y_bbvert):
    out, _ = run(X_pc, y_bbvert_pred, Y_bbvert, trace=False)
    return out
